# revision 46
# baseline (speedup 1.0000x reference)
"""Multi-head attention (N=2, S=4096, E=768, H=8 heads, D=96) + output projection,
sharded over 8 Trainium2 NeuronCores.

Sharding: data-parallel over query rows. Core i handles batch n = i//4 and query
rows (i%4)*1024 .. +1024 of that batch, attending over the batch's full K/V.
No collectives needed; the host concatenates the 8 output chunks.

Device algorithm per core (all matmuls bf16 on TensorE, f32 PSUM accumulation):
  sT[k,q]   = K_h @ Q_h^T          (scores, transposed layout: k on partitions)
  pT        = exp(sT * scale)       (split ScalarE exact / VectorE Schraudolph
                                     bit-trick, PSUM->SBUF, bf16 out)
  ctxT_aug  = V_aug^T-contract pT   (V augmented with a ones column -> row 96 of
                                     the output is the softmax denominator)
  ctxn      = ctxT * (1/denom)      (DVE + GpSimd partition-broadcast)
  Y         = sum_h ctxn_h^T @ W_h^T + b   (fc_out, row-accumulated in PSUM)

Host pre-arranges layouts (this is the sharding step): Q/K transposed to
[H, 96, S] head-major d-on-partition layout, V padded with a ones column,
fc_w transposed.
"""

import numpy as np
import ml_dtypes
import sys

for _p in ("/opt/trn_rl_repo",):
    if _p not in sys.path:
        sys.path.append(_p)

import concourse.bass as bass
import concourse.tile as tile
from concourse import bacc, mybir
from concourse.bass_utils import run_bass_kernel_spmd

F32 = mybir.dt.float32
BF16 = mybir.dt.bfloat16
I16 = mybir.dt.int16
NP_BF16 = ml_dtypes.bfloat16

N_CORES = 8
NB = 2          # batch
S = 4096        # key/value sequence length
SQ = 1024       # query rows per core
E = 768
H = 8
D = 96
KT = S // 128   # 32 k-tiles of 128
SCALE = float(np.float32(1.0) / np.sqrt(np.float32(D)))  # matches reference

# exp staging: k-tile group sizes (PSUM banks per sT tile); sum must be KT
# (small first groups -> the first exps fire sooner at startup)
EXP_GROUPS = [2, 2, 2] + [3] * 8 + [2]
assert sum(EXP_GROUPS) == KT

# ScalarE (ACT) is the bottleneck engine (~96% busy in the all-ACT version):
# offload the exp of these group indices to the Vector engine via the
# Schraudolph bit-trick: bf16(exp2(y)) ~ int16(128*(y+127) - C), computed as
# one fused tensor_scalar (mul+add, fp32 -> int16) writing into a bf16 tile
# through a bitcast AP.
DVE_GROUPS = frozenset((4, 7, 10))
# in the last block the Vector engine also runs the stage-B fc partials and
# the final norm: give it only early, smaller exp groups there
DVE_GROUPS_LAST = DVE_GROUPS
LOG2E = 1.4426950408889634
SCHRAUD_A = float(SCALE) * LOG2E * 128.0
SCHRAUD_C = 6.0  # calibrated against host sim (RNE f32->i16 convert on DVE)
SCHRAUD_B = 127.0 * 128.0 - SCHRAUD_C


def build_nc():
    nc = bacc.Bacc("TRN2", target_bir_lowering=False, debug=False)

    kT_d = nc.dram_tensor("kT", [H, D, S], BF16, kind="ExternalInput")
    qT_d = nc.dram_tensor("qT", [H, D, SQ], BF16, kind="ExternalInput")
    va_d = nc.dram_tensor("va", [H, 128, KT, D + 1], BF16, kind="ExternalInput")
    wt_d = nc.dram_tensor("wt", [E, E], BF16, kind="ExternalInput")  # fc_w.T
    bias_d = nc.dram_tensor("bias", [1, E], F32, kind="ExternalInput")
    y_d = nc.dram_tensor("y", [SQ, E], F32, kind="ExternalOutput")

    with tile.TileContext(nc) as tc:
        with (
            tc.tile_pool(name="persist", bufs=1) as persist,
            tc.tile_pool(name="pt", bufs=4) as pt_pool,
            tc.tile_pool(name="norm", bufs=2) as norm_pool,
            tc.tile_pool(name="yout", bufs=2) as y_pool,
            tc.tile_pool(name="ypart", bufs=10) as ypart_pool,
            tc.tile_pool(name="psbig", bufs=2, space="PSUM") as ps_big,
            tc.tile_pool(name="pssm", bufs=2, space="PSUM") as ps_sm,
            tc.tile_pool(name="dscratch", bufs=2, space="DRAM") as dram_pool,
        ):
            # ---- persistent SBUF tensors ----
            kT = persist.tile([D, H, S], BF16, tag="kT")          # 64 KB/part
            qT = persist.tile([D, H, SQ], BF16, tag="qT")         # 16 KB/part
            va = persist.tile([128, H, KT, D + 1], BF16, tag="va")  # 48.5 KB/part
            wt_sb = persist.tile([D, H, E], BF16, tag="wt")       # 12 KB/part
            ctxn = persist.tile([D, H, SQ], BF16, tag="ctxn")     # 16 KB/part
            bias_b = persist.tile([128, E], F32, tag="bias")      # 3 KB/part
            ones96 = persist.tile([1, D], F32, tag="ones96")
            nc.vector.memset(ones96, 1.0)
            ctx_sb = persist.tile([D, 512], F32, tag="ctxsb")     # last-block norm

            # ---- warmups ----
            # preload the exp activation-table set during the load-DMA window
            # so the first real exp doesn't pay the ~2.7us table load
            warm_act = persist.tile([1, 8], F32, tag="warm_act")
            nc.vector.memset(warm_act, 0.0)
            nc.scalar.activation(
                warm_act, warm_act, mybir.ActivationFunctionType.Exp
            )
            # a few dummy matmuls bridge PE-boot (~8us) to first-data (~10.5us)
            # so the HAM clock-gate warms before the real scores stream;
            # more than this just delays the real stream (they run cold)
            warm_w = persist.tile([D, 128], BF16, tag="warm_w")
            warm_x = persist.tile([D, 512], BF16, tag="warm_x")
            nc.vector.memset(warm_w, 0.0)
            nc.vector.memset(warm_x, 0.0)
            warm_ps = ps_sm.tile([128, 512], F32, tag="sm")
            for _ in range(6):
                nc.tensor.matmul(warm_ps, warm_w, warm_x, start=True, stop=True)

            # ---- loads ----
            # Two descriptor rings: SWDGE (gpsimd) + the sync HWDGE ring.
            # Early heads stripe across both; whole-head calls keep per-
            # partition lines at 6-8KB so the per-packet latency (~100ns)
            # amortizes (1-2KB lines cap a ring at ~170GB/s, which the PE
            # outruns). Sync goes quiet after head 2 so the norm-broadcast
            # bounces are not queued behind loads.
            nload = [0]

            def load(dst, src, eng=None):
                if eng is None:
                    eng = nc.gpsimd if nload[0] % 2 == 0 else nc.sync
                nload[0] += 1
                eng.dma_start(out=dst, in_=src)

            # head 0 in fine chunks so compute starts early
            load(qT[:, 0, 0:512], qT_d[0, :, 0:512])
            # first chunks track the first exp groups' k-tiles
            load(kT[:, 0, 0:256], kT_d[0, :, 0:256])
            load(kT[:, 0, 256:1024], kT_d[0, :, 256:1024])
            load(qT[:, 0, 512:1024], qT_d[0, :, 512:1024])
            load(kT[:, 0, 1024:2560], kT_d[0, :, 1024:2560])
            load(va[:, 0, 0:16, :], va_d[0, :, 0:16, :])
            load(kT[:, 0, 2560:], kT_d[0, :, 2560:])
            load(va[:, 0, 16:, :], va_d[0, :, 16:, :])
            for h in range(1, 3):
                load(kT[:, h, 0:2048], kT_d[h, :, 0:2048])
                load(kT[:, h, 2048:], kT_d[h, :, 2048:])
                load(qT[:, h, :], qT_d[h])
                load(va[:, h, :, :], va_d[h])
            for h in range(3, H):
                load(kT[:, h, :], kT_d[h], eng=nc.gpsimd)
                load(qT[:, h, :], qT_d[h], eng=nc.gpsimd)
                load(va[:, h, :, :], va_d[h], eng=nc.gpsimd)
            for h in range(H):
                load(wt_sb[:, h, :], wt_d[h * D:(h + 1) * D, :], eng=nc.gpsimd)
            # bias broadcast across partitions during DMA (partition step 0)
            bias_bcast = bass.AP(
                tensor=bias_d,
                offset=0,
                ap=[[0, 128], [1, E]],
            )
            nc.gpsimd.dma_start(out=bias_b, in_=bias_bcast)

            # ---- main attention loop ----
            def emit_fc(qc):
                # fc_out for query chunk qc (emitted late so it fills PE gaps)
                for qt in range(4):
                    row = qc * 512 + qt * 128
                    y_sb = y_pool.tile([128, E], F32, tag="y")
                    for half in range(2):
                        hs = half * 384
                        y_ps = ps_sm.tile([128, 384], F32, tag="sm")
                        for h in range(H):
                            nc.tensor.matmul(
                                y_ps,
                                ctxn[:, h, row:row + 128],
                                wt_sb[:, h, hs:hs + 384],
                                start=(h == 0), stop=(h == H - 1),
                            )
                        nc.vector.tensor_add(
                            y_sb[:, hs:hs + 384], y_ps, bias_b[:, hs:hs + 384]
                        )
                    # alternate store rings (sync / gpsimd are both idle here)
                    eng = nc.sync if qt % 2 == 0 else nc.gpsimd
                    eng.dma_start(out=y_d[row:row + 128, :], in_=y_sb)

            NQC = SQ // 512
            LAST_QS = (NQC - 1) * 512

            # normalize: row D of ctx_ps is the denominator.
            # Blocks 0..13: broadcast the reciprocal across partitions via a
            # DRAM round-trip (step-0 read-back) on the sync ring — idle
            # mid-run since every store now happens in the tail. The final
            # multiply is DEFERRED into the next block (the bounce takes a
            # few us; emitting the mul eagerly would block the in-order DVE
            # queue on it). The last two blocks instead use a PE rank-1
            # outer product (ones96^T @ recip): lower latency, and the tail
            # has no spare DMA slack.
            pending_norm = []
            norm_tick = [0]

            def emit_norm_recip(ctx_ps, h, qs, last_two):
                # (recip_approx is a bitwise custom-DVE op: PSUM reads
                #  corrupt it, so bounce the row through SBUF first)
                recip = norm_pool.tile([1, 512], F32, tag="recip")
                nc.vector.tensor_copy(recip, ctx_ps[D:D + 1, :])
                nc.vector.reciprocal_approx_fast(recip, recip)
                if last_two:
                    nc.vector.tensor_copy(ctx_sb, ctx_ps[0:D, :])
                    pending_norm.append(('pe', recip, None, h, qs, norm_tick[0]))
                else:
                    rdram = dram_pool.tile([1, 512], F32, tag="rd")
                    nc.sync.dma_start(out=rdram, in_=recip)
                    bcast = norm_pool.tile([D, 512], F32, tag="bcast")
                    nc.sync.dma_start(out=bcast, in_=rdram.to_broadcast([D, 512]))
                    pending_norm.append(('dma', bcast, ctx_ps, h, qs, norm_tick[0]))

            def emit_norm_finish(force=False):
                # 'dma' norms wait ~3 groups for the bounce to land so the
                # in-order DVE queue doesn't block on the mul; 'pe' norms
                # just need one group of scores in front of the bps matmul
                while pending_norm:
                    kind, op, ctx_ps, h, qs, tick = pending_norm[0]
                    age = norm_tick[0] - tick
                    if not force and age < (3 if kind == 'dma' else 1):
                        break
                    pending_norm.pop(0)
                    if kind == 'pe':
                        bps = ps_sm.tile([D, 512], F32, tag="sm")
                        nc.tensor.matmul(bps, ones96, op, start=True, stop=True)
                        nc.vector.tensor_mul(ctxn[:, h, qs:qs + 512], ctx_sb, bps)
                    else:
                        nc.vector.tensor_mul(
                            ctxn[:, h, qs:qs + 512], ctx_ps[0:D, :], op)

            # fc for the LAST chunk is split: portions are pre-accumulated
            # into SBUF partials; the tail then only needs the remaining heads.
            y_part = [None] * 8

            def emit_fc_partial(u, nheads):
                # pre-accumulate heads 0..nheads-1 of the last chunk's fc
                qt, half = divmod(u, 2)
                row = LAST_QS + qt * 128
                hs = half * 384
                yp = ypart_pool.tile([128, 384], F32, tag="ypart")
                y_part[u] = (yp, nheads)
                y_pp = ps_sm.tile([128, 384], F32, tag="sm")
                for h in range(nheads):
                    nc.tensor.matmul(
                        y_pp,
                        ctxn[:, h, row:row + 128],
                        wt_sb[:, h, hs:hs + 384],
                        start=(h == 0), stop=(h == nheads - 1),
                    )
                nc.vector.tensor_add(yp, y_pp, bias_b[:, hs:hs + 384])

            def emit_fc_partial_b(u):
                # second partial stage (runs inside the last block): fold
                # heads nheads..H-2 into the unit's SBUF partial so only the
                # last head's matmul + one add + store remain in the tail
                qt, half = divmod(u, 2)
                row = LAST_QS + qt * 128
                hs = half * 384
                yp, nheads = y_part[u]
                if nheads >= H - 1:
                    return
                ypb = ypart_pool.tile([128, 384], F32, tag="ypart")
                y_pb = ps_sm.tile([128, 384], F32, tag="sm")
                for h in range(nheads, H - 1):
                    nc.tensor.matmul(
                        y_pb,
                        ctxn[:, h, row:row + 128],
                        wt_sb[:, h, hs:hs + 384],
                        start=(h == nheads), stop=(h == H - 2),
                    )
                nc.vector.tensor_add(ypb, y_pb, yp)
                y_part[u] = (ypb, H - 1)

            def emit_fc_final():
                # tail: per half-unit just the last head's matmul, one add,
                # and a store fanned out over all three DMA rings (scalar's
                # HWDGE ring is safe here — no ACTIVATEs follow)
                store_rings = [nc.sync, nc.scalar, nc.gpsimd, nc.scalar]
                for qt in range(4):
                    row = LAST_QS + qt * 128
                    y_sb = y_pool.tile([128, E], F32, tag="y")
                    for half in range(2):
                        hs = half * 384
                        yp, nheads = y_part[qt * 2 + half]
                        y_ps2 = ps_sm.tile([128, 384], F32, tag="sm")
                        for h in range(nheads, H):
                            nc.tensor.matmul(
                                y_ps2,
                                ctxn[:, h, row:row + 128],
                                wt_sb[:, h, hs:hs + 384],
                                start=(h == nheads), stop=(h == H - 1),
                            )
                        nc.vector.tensor_add(y_sb[:, hs:hs + 384], y_ps2, yp)
                    store_rings[qt].dma_start(out=y_d[row:row + 128, :], in_=y_sb)

            # Software-pipelined ctx matmuls: lag the exp stream by 2 groups so
            # the in-order PE queue never waits on an exp at block boundaries.
            pend = []   # (ctx_ps, h, qs, kt0, pt, is_last_group)

            def flush_one():
                c_ps, c_h, c_qs, c_kt0, c_pt, c_last = pend.pop(0)
                _emit_ctx(nc, c_ps, va, (c_kt0, c_pt), c_h)
                if c_last:
                    # (a DRAM-bounce broadcast for mid-run blocks was tried
                    # here and measured ~7us slower end-to-end)
                    emit_norm_recip(c_ps, c_h, c_qs, True)

            for qc in range(NQC):
                qs = qc * 512
                for h in range(H):
                    ctx_ps = ps_sm.tile([D + 1, 512], F32, tag="sm")
                    partials = None
                    partials_b = None
                    if qc == NQC - 1 and h == H - 4:
                        partials = ([0, 1], H - 4)
                    elif qc == NQC - 1 and h == H - 3:
                        partials = ([2, 3, 4], H - 3)
                    elif qc == NQC - 1 and h == H - 2:
                        partials = ([5, 6, 7], H - 2)

                    kt0 = 0
                    for gi, g in enumerate(EXP_GROUPS):
                        norm_tick[0] += 1
                        sT = ps_big.tile([128, g * 512], F32, tag="sT")
                        for j in range(g):
                            kt = kt0 + j
                            nc.tensor.matmul(
                                sT[:, j * 512:(j + 1) * 512],
                                kT[:, h, kt * 128:(kt + 1) * 128],
                                qT[:, h, qs:qs + 512],
                                start=True, stop=True,
                            )
                        pt = pt_pool.tile([128, g * 512], BF16, tag="pt")
                        dve_set = (DVE_GROUPS_LAST
                                   if (qc == NQC - 1 and h == H - 1)
                                   else DVE_GROUPS)
                        if gi in dve_set:
                            nc.vector.tensor_scalar(
                                pt.bitcast(I16), sT,
                                SCHRAUD_A, SCHRAUD_B,
                                mybir.AluOpType.mult, mybir.AluOpType.add,
                            )
                        else:
                            nc.scalar.activation(
                                pt, sT, mybir.ActivationFunctionType.Exp,
                                scale=SCALE,
                            )
                        pend.append(
                            (ctx_ps, h, qs, kt0, pt, gi == len(EXP_GROUPS) - 1)
                        )
                        while len(pend) > 3:
                            flush_one()
                        emit_norm_finish()
                        kt0 += g
                        if partials is not None and gi in (3, 6, 9):
                            units, nheads = partials
                            ui = gi // 3 - 1
                            if ui < len(units):
                                # reads ctxn of heads < nheads: the previous
                                # head's deferred norm must land first
                                emit_norm_finish(force=True)
                                emit_fc_partial(units[ui], nheads)
                        if partials_b is not None and gi in partials_b:
                            for u in partials_b[gi]:
                                emit_fc_partial_b(u)
                if qc > 0:
                    # fc reads ctxn of every head in chunk qc-1: drain the ctx
                    # pipeline (forces the last head's recip) and finish all
                    # deferred norms first
                    while pend:
                        flush_one()
                    emit_norm_finish(force=True)
                    emit_fc(qc - 1)
            while pend:
                flush_one()
            emit_norm_finish(force=True)
            emit_fc_final()

    nc.finalize()
    return nc


def _emit_ctx(nc, ctx_ps, va, pending, h):
    kt0, pt = pending
    g = pt.shape[1] // 512
    for j in range(g):
        kt = kt0 + j
        nc.tensor.matmul(
            ctx_ps,
            va[:, h, kt, :],
            pt[:, j * 512:(j + 1) * 512],
            start=(kt == 0), stop=(kt == KT - 1),
        )


def _prep_inputs(values, keys, query, fc_w, fc_b):
    """Build per-core input maps (host-side sharding + layout + bf16 cast)."""
    values = np.ascontiguousarray(values, dtype=np.float32)
    keys = np.ascontiguousarray(keys, dtype=np.float32)
    query = np.ascontiguousarray(query, dtype=np.float32)
    wt = np.ascontiguousarray(np.asarray(fc_w, dtype=np.float32).T.astype(NP_BF16))
    bias = np.ascontiguousarray(np.asarray(fc_b, dtype=np.float32).reshape(1, E))

    per_batch = []
    for n in range(NB):
        # K -> [H, D, S]
        kTn = np.ascontiguousarray(
            keys[n].reshape(S, H, D).transpose(1, 2, 0).astype(NP_BF16))
        # V -> [H, 128, KT, D+1] with ones in the last column
        # (partition-contiguous: per head, each of the 128 partitions reads
        #  KT*(D+1) contiguous values -> large DMA descriptors)
        van = np.empty((H, 128, KT, D + 1), dtype=NP_BF16)
        van[..., :D] = values[n].reshape(KT, 128, H, D).transpose(2, 1, 0, 3)
        van[..., D] = 1.0
        per_batch.append((kTn, van))

    in_maps = []
    for core in range(N_CORES):
        n = core // (N_CORES // NB)
        qi = core % (N_CORES // NB)
        qrows = query[n, qi * SQ:(qi + 1) * SQ]
        qTn = np.ascontiguousarray(
            qrows.reshape(SQ, H, D).transpose(1, 2, 0).astype(NP_BF16))
        kTn, van = per_batch[n]
        in_maps.append({
            "kT": kTn, "qT": qTn, "va": van, "wt": wt, "bias": bias,
        })
    return in_maps


def _assemble(results):
    y = np.empty((NB, S, E), dtype=np.float32)
    for core in range(N_CORES):
        n = core // (N_CORES // NB)
        qi = core % (N_CORES // NB)
        y[n, qi * SQ:(qi + 1) * SQ] = results[core]["y"]
    return y


def run(values, keys, query, fc_w, fc_b, **spmd_kwargs):
    nc = build_nc()
    in_maps = _prep_inputs(values, keys, query, fc_w, fc_b)
    res = run_bass_kernel_spmd(nc, in_maps, core_ids=list(range(N_CORES)),
                               **spmd_kwargs)
    return _assemble(res.results), res


def kernel(values, keys, query, fc_w, fc_b):
    y, _ = run(values, keys, query, fc_w, fc_b)
    return y

